# revision 42
# baseline (speedup 1.0000x reference)
"""Multi-head causal self-attention with RoPE on 8 TRN2 NeuronCores.

Sharding: tensor-parallel over heads. Each core owns 2 of the 16 heads:
it gets the matching rows of W_qkv and columns of W_o (host-sliced,
host-transposed, bf16-cast), computes a full [B*L, D] partial output, and
the host sums the 8 partials (the o_proj all-reduce).

Per-core pipeline (all matmuls bf16/fp16 with fp32 PSUM accumulation):
  phase A: x --DMA-xbar-transpose--> xT chunks; xT (stationary) @ WqkvT
    (moving) -> q,k,v natural [t, e]; RoPE on q,k; PE-transpose -> qT,kT.
  phase B: per (b, qc) with both heads block-interleaved and one-group
    software pipelining: S^T = kT^T @ qT; p^T = exp(scale*S^T) (fp16);
    causal diag masks on gpsimd; row-sum l accumulated as fp16 exp-block
    sums on the DVE + one small ones-matmul per (h, qc); y^T = v^T @ p^T;
    y^T * exp(-ln l) -> yT_sb.
  phase C: o_proj decomposed into (lt, ec) mini-units woven into phase B
    one q-chunk behind, storing fp32 partials DMA-direct from PSUM.
"""

import sys

if "/opt/trn_rl_repo" not in sys.path:
    sys.path.insert(0, "/opt/trn_rl_repo")

import math

import ml_dtypes
import numpy as np

import concourse.bass as bass
import concourse.mybir as mybir
import concourse.tile as tile
from concourse.bass_utils import run_bass_kernel_spmd
from concourse.vector_clock import ScopedClock

BF16 = ml_dtypes.bfloat16
FP16 = np.float16
FP32 = mybir.dt.float32
BF = mybir.dt.bfloat16
F16 = mybir.dt.float16

ROPE_THETA = 10000.0


def _split_multi_waits(nc):
    """This container's walrus build rejects >1 sync wait per instruction.
    Hoist all but one wait of each instruction onto same-engine NOPs placed
    immediately before it (same-engine program order makes this equivalent)."""
    for f in nc.m.functions:
        for bb in f.blocks:
            insts = bb.instructions
            if not any(
                i.sync_info is not None and len(i.sync_info.on_wait) > 1
                for i in insts
            ):
                continue
            new = []
            for inst in insts:
                si = inst.sync_info
                if si is not None and len(si.on_wait) > 1:
                    waits = list(si.on_wait)
                    si.on_wait.clear()
                    si.on_wait.append(waits[0])
                    for idx, w in enumerate(waits[1:]):
                        new.append(
                            mybir.InstNoOp(
                                name=f"{inst.name}-sw{idx}",
                                engine=inst.engine,
                                bass_nofuse=True,
                                sync_info=mybir.SyncInfo(on_wait=[w], on_update=[]),
                            )
                        )
                new.append(inst)
            bb.instructions = new


class TileContextSplitDrain(tile.TileContext):
    """TileContext adapted to this walrus build's 1-sync-wait-per-instruction
    limit: split the tail drain's waits and run _split_multi_waits over the
    whole scheduled program."""

    def _drain_and_barrier(self, tick_clock, wait_clock):
        _split_multi_waits(self.nc)
        drain_inst = self.nc.sync.drain()
        wait_clock.add_sem_waits(
            drain_inst.ins, ScopedClock({None: tick_clock.global_clock})
        )
        waits = list(drain_inst.ins.sync_info.on_wait)
        if len(waits) > 1:
            drain_inst.ins.sync_info.on_wait.clear()
            drain_inst.ins.sync_info.on_wait.append(waits[0])
            for w in waits[1:]:
                nop = self.nc.sync.nop(nofuse=True)
                if nop.ins.sync_info is None:
                    nop.ins.sync_info = mybir.SyncInfo(on_update=[], on_wait=[])
                nop.ins.sync_info.on_wait.append(w)

        self.nc.all_engine_barrier()
        assert self.sems is not None
        popped = self.nc._tile_sem_poison_stack.pop()
        assert popped is self._sem_poison
        self.nc.clear_and_free_semaphores(list(self.sems.allocated().values()))
        self.nc.all_engine_barrier()


def _bcast_mid(ap, rep):
    """[128, N] AP -> [128, rep, N] AP with a stride-0 middle dim."""
    return bass.AP(ap.tensor, ap.offset, [ap.ap[0], [0, rep], ap.ap[1]])


def build_core_kernel(B, L, D, HL, dh=128, TCH=512, QC=512):
    """One core's kernel: HL local heads over the full [B*L, D] input."""
    T = B * L
    DT = D // 128          # d-tiles
    LT = L // 128          # l-tiles per batch
    EQK = 2 * HL * dh      # q+k columns in wqkvT
    EV = HL * dh           # v columns
    NCH = T // TCH         # t-chunks
    TPC = TCH // 128       # t-tiles per chunk
    NQC = L // QC          # q-chunks per (b, h)
    NDIAG = QC // 128
    scale = 1.0 / math.sqrt(dh)

    nc = bass.Bass("TRN2", target_bir_lowering=False)
    xbT = nc.dram_tensor("xbT", [D, T], BF, kind="ExternalInput")
    wqkvT = nc.dram_tensor("wqkvT", [D, EQK + EV], BF, kind="ExternalInput")
    woT = nc.dram_tensor("woT", [EV, D], BF, kind="ExternalInput")
    cosn = nc.dram_tensor("cosn", [128, LT, dh], FP32, kind="ExternalInput")
    sinn = nc.dram_tensor("sinn", [128, LT, dh], FP32, kind="ExternalInput")
    masks = nc.dram_tensor("masks", [128, 128], F16, kind="ExternalInput")
    ident = nc.dram_tensor("ident", [128, 128], BF, kind="ExternalInput")
    out = nc.dram_tensor("out", [T, D], BF, kind="ExternalOutput")

    with TileContextSplitDrain(nc) as tc:
        with (
            tc.tile_pool(name="static", bufs=1) as st,
            tc.tile_pool(name="xt", bufs=2) as xt_pool,
            tc.tile_pool(name="ropef", bufs=2) as ropef,
            tc.tile_pool(name="qknat", bufs=3) as qknat_pool,
            tc.tile_pool(name="pt", bufs=8) as pt_pool,
            tc.tile_pool(name="laccp", bufs=4) as laccp,
            tc.tile_pool(name="small", bufs=2) as small,
            tc.tile_pool(name="ostage", bufs=3) as ostage_pool,
            tc.tile_pool(name="psum_proj", bufs=2, space="PSUM") as ps_proj,
            tc.tile_pool(name="psum_trs", bufs=3, space="PSUM") as ps_trs,
            tc.tile_pool(name="psum_y", bufs=3, space="PSUM") as ps_y,
        ):
            # --- static loads, ordered so the first qk matmul can start
            # as early as possible: wq (4 dt-group DMAs, scalar ring)
            # races x chunk 0 (4 dt-group DMAs, sync ring) ---
            # startup: interleave wq / x-chunk-0 / early rope tables across
            # both DMA rings so the first tile is never starved
            wqv = wqkvT.rearrange("(dt p) e -> p dt e", p=128)
            xbT3 = xbT.rearrange("(dt p) t -> p dt t", p=128)
            wq_sb = st.tile([128, DT, EQK + EV], BF)
            xT_c0 = xt_pool.tile([128, DT, TCH], BF, name="xT_c")
            cos_sb = st.tile([128, LT, dh], FP32)
            sin_sb = st.tile([128, LT, dh], FP32)
            for g in range(8):
                eng = nc.sync if g % 2 == 0 else nc.gpsimd
                eng.dma_start(
                    xT_c0[:, 2 * g:2 * g + 2, :], xbT3[:, 2 * g:2 * g + 2, 0:TCH]
                )
            nc.scalar.dma_start(wq_sb[:, 0:4, :], wqv[:, 0:4, :])
            nc.scalar.dma_start(cos_sb[:, 0:4, :], cosn[:, 0:4, :])
            nc.scalar.dma_start(sin_sb[:, 0:4, :], sinn[:, 0:4, :])
            nc.scalar.dma_start(wq_sb[:, 4:8, :], wqv[:, 4:8, :])
            nc.scalar.dma_start(wq_sb[:, 8:12, :], wqv[:, 8:12, :])
            nc.scalar.dma_start(wq_sb[:, 12:16, :], wqv[:, 12:16, :])
            ident_sb = st.tile([128, 128], BF)
            nc.scalar.dma_start(ident_sb[:], ident[:])
            nc.scalar.dma_start(cos_sb[:, 4:LT, :], cosn[:, 4:LT, :])
            nc.scalar.dma_start(sin_sb[:, 4:LT, :], sinn[:, 4:LT, :])
            masks_sb = st.tile([128, 128], F16)
            nc.scalar.dma_start(masks_sb[:], masks[:])
            wo_sb = st.tile([128, HL, D], BF)
            nc.scalar.dma_start(wo_sb[:], woT.rearrange("(h p) e -> p h e", p=128))
            ones_mat = st.tile([128, 128], F16)
            nc.vector.memset(ones_mat[:], 1.0)

            qT_sb = st.tile([128, HL, B, L], BF)
            kT_sb = st.tile([128, HL, B, L], BF)
            v_sb = st.tile([128, T // 128, EV], F16)
            yT_sb = st.tile([128, HL, B, L], BF)

            # --- phase A: qkv projection + rope + q/k transpose ---
            copy_state = [0]

            def emit_a_chunk(tch, pump_n=0):
                copy_flip = copy_state[0]
                b = (tch * TCH) // L
                l0 = (tch * TCH) % L
                if tch == 0:
                    xT_c = xT_c0    # loaded in the startup interleave
                else:
                    xT_c = xt_pool.tile([128, DT, TCH], BF, name="xT_c")
                    csl = slice(tch * TCH, (tch + 1) * TCH)
                    # dt-splits keep 1KB DRAM rows per descriptor; early
                    # chunks split across two rings for bandwidth
                    for g in range(2):
                        eng = nc.sync if (g == 0 or tch >= 4) else nc.gpsimd
                        eng.dma_start(
                            out=xT_c[:, 8 * g:8 * g + 8, :],
                            in_=xbT3[:, 8 * g:8 * g + 8, csl],
                        )
                for tt in range(TPC):
                    lt = (l0 + tt * 128) // 128
                    tstat = [xT_c[:, dt, tt * 128:(tt + 1) * 128] for dt in range(DT)]
                    # q,k natural [t, e]
                    ps_qk = ps_proj.tile([128, EQK], FP32, tag="proj")
                    for dt in range(DT):
                        nc.tensor.matmul(
                            ps_qk[:], tstat[dt], wq_sb[:, dt, 0:EQK],
                            start=(dt == 0), stop=(dt == DT - 1),
                        )
                    # v natural [t, e] (PE does v while rope consumes ps_qk)
                    ps_v = ps_proj.tile([128, EV], FP32, tag="proj")
                    for dt in range(DT):
                        nc.tensor.matmul(
                            ps_v[:], tstat[dt], wq_sb[:, dt, EQK:EQK + EV],
                            start=(dt == 0), stop=(dt == DT - 1),
                        )
                    # rope: the pair swap is folded into tB's read as a
                    # negative-stride view of the psum (no staging copies)
                    cos_b = _bcast_mid(cos_sb[:, lt, :], 2 * HL)
                    sin4 = bass.AP(
                        sin_sb.tensor,
                        sin_sb[:, lt, :].offset,
                        [sin_sb.ap[0], [0, 2 * HL], [2, dh // 2], [1, 2]],
                    )
                    ps4 = ps_qk.rearrange("p (h c two) -> p h c two", h=2 * HL, two=2)
                    sw4 = bass.AP(
                        ps_qk.tensor,
                        ps_qk.offset + 1,
                        [ps_qk.ap[0], [dh, 2 * HL], [2, dh // 2], [-1, 2]],
                    )
                    tA = ropef.tile([128, EQK], FP32, tag="tA")
                    tB = ropef.tile([128, EQK], FP32, tag="tB")
                    nc.vector.tensor_mul(
                        tA.rearrange("p (h e) -> p h e", e=dh),
                        ps_qk.rearrange("p (h e) -> p h e", e=dh), cos_b
                    )
                    nc.vector.tensor_mul(
                        tB.rearrange("p (h c two) -> p h c two", h=2 * HL, two=2),
                        sw4, sin4
                    )
                    qk_nat = qknat_pool.tile([128, EQK], BF)
                    nc.vector.tensor_add(qk_nat[:], tA[:], tB[:])
                    # transpose q,k slices -> qT/kT (PE reaches here after the
                    # v matmuls, by which time qk_nat is ready)
                    ps_t = ps_trs.tile([128, EQK], BF, tag="trs")
                    for j in range(2 * HL):
                        nc.tensor.transpose(
                            ps_t[:, j * 128:(j + 1) * 128],
                            qk_nat[:, j * 128:(j + 1) * 128],
                            ident_sb[:],
                        )
                    lsl = slice(l0 + tt * 128, l0 + (tt + 1) * 128)
                    q_dst = qT_sb[:, :, b, lsl]
                    k_dst = kT_sb[:, :, b, lsl]
                    q_src = ps_t[:, 0:HL * 128].rearrange("p (h t) -> p h t", h=HL)
                    k_src = ps_t[:, HL * 128:2 * HL * 128].rearrange(
                        "p (h t) -> p h t", h=HL
                    )
                    nc.scalar.activation(
                        q_dst, q_src, mybir.ActivationFunctionType.Copy
                    )
                    nc.vector.tensor_copy(k_dst, k_src)
                    copy_flip += 1
                    nc.scalar.activation(
                        v_sb[:, tch * TPC + tt, :], ps_v[:],
                        mybir.ActivationFunctionType.Copy,
                    )
                    emit_c_mini()
                    if pump_n:
                        pump(pump_n)
                copy_state[0] = copy_flip

            # --- phases B and C, woven: attention per (b, qc) with both
            # heads block-interleaved; o_proj mini-units (b, lt, ec) for the
            # previous q-chunk fill PE slack and spread the output stores ---
            c_queue = []       # pending o_proj minis: (b, lt, ec)
            c_stage = [None]   # current per-lt staging tile
            c_tail_lt = [None]  # lt whose staging began in drain mode

            def emit_c_mini(tail=False):
                if not c_queue:
                    return
                b, lt, ec = c_queue.pop(0)
                if tail and ec == 0:
                    c_tail_lt[0] = (b, lt)
                tail = tail and c_tail_lt[0] == (b, lt)
                ttg = b * LT + lt
                ps_o = ps_proj.tile([128, 512], FP32, tag="proj")
                for h in range(HL):
                    nc.tensor.matmul(
                        ps_o[:],
                        yT_sb[:, h, b, lt * 128:(lt + 1) * 128],
                        wo_sb[:, h, ec * 512:(ec + 1) * 512],
                        start=(h == 0), stop=(h == HL - 1),
                    )
                if ec == 0:
                    c_stage[0] = ostage_pool.tile([128, D // 512, 512], BF, name="ost")
                ost = c_stage[0]
                # casts split scalar/vector (gpsimd cannot read PSUM);
                # vector-heavy in the tail where scalar still runs exps
                if ec in (0, 2) or (tail and ec == 1):
                    nc.vector.tensor_copy(ost[:, ec, :], ps_o[:])
                else:
                    nc.scalar.activation(
                        ost[:, ec, :], ps_o[:], mybir.ActivationFunctionType.Copy
                    )
                if tail:
                    # in the drain, store each 512-wide slice as soon as its
                    # cast lands instead of batching the whole row
                    nc.sync.dma_start(
                        out[ttg * 128:(ttg + 1) * 128, ec * 512:(ec + 1) * 512],
                        ost[:, ec, :],
                    )
                elif ec == D // 512 - 1:
                    nc.sync.dma_start(
                        out[ttg * 128:(ttg + 1) * 128, :],
                        ost.rearrange("p e c -> p (e c)"),
                    )

            def phase_b_qc(b, qc):
                nk = (qc + 1) * NDIAG
                ps_yt = [
                    ps_y.tile([128, QC], FP32, name=f"ps_yt{h}", tag="y")
                    for h in range(HL)
                ]
                lacc = [
                    laccp.tile([128, QC], F16, name=f"lacc{h}", tag="lacc")
                    for h in range(HL)
                ]
                pT_blk = [[None] * nk for _ in range(HL)]
                cs_blk = [None] * nk

                def s_exp(h, kb):
                    q_lo = max(0, kb * 128 - qc * QC)
                    cs = slice(q_lo, QC)
                    cs_blk[kb] = (q_lo, cs)
                    qmov = qT_sb[:, h, b, qc * QC + q_lo:(qc + 1) * QC]
                    ps_s = ps_trs.tile([128, QC], FP32, tag="trs")
                    nc.tensor.matmul(
                        ps_s[:, cs],
                        kT_sb[:, h, b, kb * 128:(kb + 1) * 128],
                        qmov,
                        start=True, stop=True,
                    )
                    pT = pt_pool.tile([128, QC], F16)
                    nc.scalar.activation(
                        pT[:, cs], ps_s[:, cs],
                        mybir.ActivationFunctionType.Exp, scale=scale,
                    )
                    # launch the diag mask on gpsimd right away: its Q7
                    # launch+sem latency hides behind the one-group lag
                    if kb >= NDIAG * qc:
                        nc.gpsimd.tensor_mul(
                            pT[:, q_lo:q_lo + 128],
                            pT[:, q_lo:q_lo + 128],
                            masks_sb[:],
                        )
                    pT_blk[h][kb] = pT

                def post(h, kb):
                    q_lo, cs = cs_blk[kb]
                    pT = pT_blk[h][kb]
                    if kb == 0:
                        nc.vector.tensor_copy(lacc[h][:], pT[:])
                    else:
                        nc.vector.tensor_add(lacc[h][:, cs], lacc[h][:, cs], pT[:, cs])
                    nc.tensor.matmul(
                        ps_yt[h][:, cs],
                        v_sb[:, b * LT + kb, h * dh:(h + 1) * dh],
                        pT[:, cs],
                        start=(kb == 0), stop=(kb == nk - 1),
                    )

                # software pipeline: o_proj minis pad the gap between the
                # S matmuls for kb and the PV matmuls for kb-1, giving the
                # exp/mask chain time to produce p^T
                for kb in range(nk):
                    for h in range(HL):
                        s_exp(h, kb)
                    emit_c_mini()
                    emit_c_mini()
                    if kb >= 1:
                        for h in range(HL):
                            post(h, kb - 1)
                    yield
                for h in range(HL):
                    post(h, nk - 1)
                # normalization: l = ones^T @ lacc (replicated across
                # partitions), 1/l = exp(-ln l), yT = ps_yt * (1/l)
                for h in range(HL):
                    emit_c_mini()
                    ps_lt = ps_trs.tile([128, QC], FP32, tag="trs", name="ps_lt")
                    nc.tensor.matmul(
                        ps_lt[:], ones_mat[:], lacc[h][:], start=True, stop=True
                    )
                    emit_c_mini()
                    lnl = small.tile([128, QC], FP32, tag="lnl")
                    nc.scalar.activation(
                        lnl[:], ps_lt[:], mybir.ActivationFunctionType.Ln
                    )
                    invb = small.tile([128, QC], FP32, tag="invb")
                    nc.scalar.activation(
                        invb[:], lnl[:],
                        mybir.ActivationFunctionType.Exp, scale=-1.0,
                    )
                    nc.vector.tensor_mul(
                        yT_sb[:, h, b, qc * QC:(qc + 1) * QC], ps_yt[h][:], invb[:]
                    )
                    yield
                # this q-chunk's o_proj minis become available for weaving
                for lt in range(qc * NDIAG, (qc + 1) * NDIAG):
                    for ec in range(D // 512):
                        c_queue.append((b, lt, ec))

            # --- global weave: attention emission is a generator pumped a
            # few steps after every phase-A tile, so A matmuls fill the
            # S->exp->mask->lacc->PV latency chain and the scalar/vector
            # load spreads over the whole kernel. Unit (b, qc) is gated on
            # the A chunk holding its data being fully emitted. ---
            chunks_emitted = [0]

            def b_stream():
                for b in range(B):
                    for qc in range(NQC):
                        yield (1 + qc) if b == 0 else (5 + qc)  # chunks needed
                        yield from phase_b_qc(b, qc)

            gen = b_stream()
            gate = [0]
            done = [False]

            def pump(n):
                for _ in range(n):
                    if done[0] or chunks_emitted[0] < gate[0]:
                        return
                    try:
                        g = next(gen)
                    except StopIteration:
                        done[0] = True
                        return
                    if isinstance(g, int):
                        gate[0] = g

            for tch in range(NCH):
                emit_a_chunk(tch, pump_n=4 if tch >= 1 else 0)
                chunks_emitted[0] = tch + 1
            while not done[0]:
                pump(4)
                emit_c_mini()
                emit_c_mini()
            while c_queue:
                emit_c_mini(tail=True)
    return nc


def _rope_tables(L, dh, LT):
    inv_freq = 1.0 / (ROPE_THETA ** (np.arange(0, dh, 2, dtype=np.float32) / dh))
    ang = np.arange(L, dtype=np.float32)[:, None] * inv_freq[None, :]  # [L, dh/2]
    cos = np.repeat(np.cos(ang), 2, axis=-1)                          # [L, dh]
    sin = np.repeat(np.sin(ang), 2, axis=-1)
    sgn = np.where(np.arange(dh) % 2 == 0, -1.0, 1.0).astype(np.float32)
    sinn = sin * sgn[None, :]
    # [L, dh] -> [128, LT, dh] with partition = l % 128
    cosn = np.ascontiguousarray(
        cos.reshape(LT, 128, dh).transpose(1, 0, 2)
    ).astype(np.float32)
    sinn = np.ascontiguousarray(
        sinn.reshape(LT, 128, dh).transpose(1, 0, 2)
    ).astype(np.float32)
    return cosn, sinn


def make_in_maps(x, W_qkv, W_o, n_cores=8, H=16):
    B, L, D = x.shape
    T = B * L
    dh = D // H
    HL = H // n_cores
    LT = L // 128
    xbfT = np.ascontiguousarray(x.reshape(T, D).T).astype(BF16)
    cosn, sinn = _rope_tables(L, dh, LT)
    p = np.arange(128)[:, None]
    f = np.arange(128)[None, :]
    mask = (p <= f).astype(FP16)
    identity = np.eye(128, dtype=BF16)
    in_maps = []
    for c in range(n_cores):
        r0 = c * HL * dh
        r1 = (c + 1) * HL * dh
        wl = np.concatenate(
            [W_qkv[r0:r1], W_qkv[D + r0:D + r1], W_qkv[2 * D + r0:2 * D + r1]], axis=0
        )
        wqkvT = np.ascontiguousarray(wl.T).astype(BF16)
        woT = np.ascontiguousarray(W_o[:, r0:r1].T).astype(BF16)
        in_maps.append(
            {
                "xbT": xbfT,
                "wqkvT": wqkvT,
                "woT": woT,
                "cosn": cosn,
                "sinn": sinn,
                "masks": mask,
                "ident": identity,
            }
        )
    return in_maps


_NC_CACHE = {}


def _get_nc(B, L, D, HL):
    key = (B, L, D, HL)
    if key not in _NC_CACHE:
        _NC_CACHE[key] = build_core_kernel(B, L, D, HL)
    return _NC_CACHE[key]


def kernel(x, W_qkv, W_o, trace=False):
    x = np.asarray(x)
    W_qkv = np.asarray(W_qkv)
    W_o = np.asarray(W_o)
    B, L, D = x.shape
    n_cores, H = 8, 16
    HL = H // n_cores
    nc = _get_nc(B, L, D, HL)
    in_maps = make_in_maps(x, W_qkv, W_o, n_cores=n_cores, H=H)
    res = run_bass_kernel_spmd(
        nc, in_maps, core_ids=list(range(n_cores)), trace=trace
    )
    acc = np.zeros((B * L, D), dtype=np.float64)
    for r in res.results:
        acc += r["out"].astype(np.float64)
    out = acc.astype(np.float32).reshape(B, L, D)
    if trace:
        return out, res
    return out


# revision 46
# speedup vs baseline: 1.0216x; 1.0216x over previous
"""Multi-head causal self-attention with RoPE on 8 TRN2 NeuronCores.

Sharding: tensor-parallel over heads. Each core owns 2 of the 16 heads:
it gets the matching rows of W_qkv and columns of W_o (host-sliced,
host-transposed, bf16-cast), computes a full [B*L, D] partial output, and
the host sums the 8 partials (the o_proj all-reduce).

Per-core pipeline (all matmuls bf16/fp16 with fp32 PSUM accumulation):
  phase A: x --DMA-xbar-transpose--> xT chunks; xT (stationary) @ WqkvT
    (moving) -> q,k,v natural [t, e]; RoPE on q,k; PE-transpose -> qT,kT.
  phase B: per (b, qc) with both heads block-interleaved and one-group
    software pipelining: S^T = kT^T @ qT; p^T = exp(scale*S^T) (fp16);
    causal diag masks on gpsimd; row-sum l accumulated as fp16 exp-block
    sums on the DVE + one small ones-matmul per (h, qc); y^T = v^T @ p^T;
    y^T * exp(-ln l) -> yT_sb.
  phase C: o_proj decomposed into (lt, ec) mini-units woven into phase B
    one q-chunk behind, storing fp32 partials DMA-direct from PSUM.
"""

import sys

if "/opt/trn_rl_repo" not in sys.path:
    sys.path.insert(0, "/opt/trn_rl_repo")

import math

import ml_dtypes
import numpy as np

import concourse.bass as bass
import concourse.mybir as mybir
import concourse.tile as tile
from concourse.bass_utils import run_bass_kernel_spmd
from concourse.vector_clock import ScopedClock

BF16 = ml_dtypes.bfloat16
FP16 = np.float16
FP32 = mybir.dt.float32
BF = mybir.dt.bfloat16
F16 = mybir.dt.float16

ROPE_THETA = 10000.0


def _split_multi_waits(nc):
    """This container's walrus build rejects >1 sync wait per instruction.
    Hoist all but one wait of each instruction onto same-engine NOPs placed
    immediately before it (same-engine program order makes this equivalent)."""
    for f in nc.m.functions:
        for bb in f.blocks:
            insts = bb.instructions
            if not any(
                i.sync_info is not None and len(i.sync_info.on_wait) > 1
                for i in insts
            ):
                continue
            new = []
            for inst in insts:
                si = inst.sync_info
                if si is not None and len(si.on_wait) > 1:
                    waits = list(si.on_wait)
                    si.on_wait.clear()
                    si.on_wait.append(waits[0])
                    for idx, w in enumerate(waits[1:]):
                        new.append(
                            mybir.InstNoOp(
                                name=f"{inst.name}-sw{idx}",
                                engine=inst.engine,
                                bass_nofuse=True,
                                sync_info=mybir.SyncInfo(on_wait=[w], on_update=[]),
                            )
                        )
                new.append(inst)
            bb.instructions = new


class TileContextSplitDrain(tile.TileContext):
    """TileContext adapted to this walrus build's 1-sync-wait-per-instruction
    limit: split the tail drain's waits and run _split_multi_waits over the
    whole scheduled program."""

    def _drain_and_barrier(self, tick_clock, wait_clock):
        _split_multi_waits(self.nc)
        drain_inst = self.nc.sync.drain()
        wait_clock.add_sem_waits(
            drain_inst.ins, ScopedClock({None: tick_clock.global_clock})
        )
        waits = list(drain_inst.ins.sync_info.on_wait)
        if len(waits) > 1:
            drain_inst.ins.sync_info.on_wait.clear()
            drain_inst.ins.sync_info.on_wait.append(waits[0])
            for w in waits[1:]:
                nop = self.nc.sync.nop(nofuse=True)
                if nop.ins.sync_info is None:
                    nop.ins.sync_info = mybir.SyncInfo(on_update=[], on_wait=[])
                nop.ins.sync_info.on_wait.append(w)

        self.nc.all_engine_barrier()
        assert self.sems is not None
        popped = self.nc._tile_sem_poison_stack.pop()
        assert popped is self._sem_poison
        self.nc.clear_and_free_semaphores(list(self.sems.allocated().values()))
        self.nc.all_engine_barrier()


def _bcast_mid(ap, rep):
    """[128, N] AP -> [128, rep, N] AP with a stride-0 middle dim."""
    return bass.AP(ap.tensor, ap.offset, [ap.ap[0], [0, rep], ap.ap[1]])


def build_core_kernel(B, L, D, HL, dh=128, TCH=512, QC=512):
    """One core's kernel: HL local heads over the full [B*L, D] input."""
    T = B * L
    DT = D // 128          # d-tiles
    LT = L // 128          # l-tiles per batch
    EQK = 2 * HL * dh      # q+k columns in wqkvT
    EV = HL * dh           # v columns
    NCH = T // TCH         # t-chunks
    TPC = TCH // 128       # t-tiles per chunk
    NQC = L // QC          # q-chunks per (b, h)
    NDIAG = QC // 128
    scale = 1.0 / math.sqrt(dh)

    nc = bass.Bass("TRN2", target_bir_lowering=False)
    xbT = nc.dram_tensor("xbT", [D, T], BF, kind="ExternalInput")
    wqkvT = nc.dram_tensor("wqkvT", [D, EQK + EV], BF, kind="ExternalInput")
    woT = nc.dram_tensor("woT", [EV, D], BF, kind="ExternalInput")
    cosn = nc.dram_tensor("cosn", [128, LT, dh], FP32, kind="ExternalInput")
    sinn = nc.dram_tensor("sinn", [128, LT, dh], FP32, kind="ExternalInput")
    masks = nc.dram_tensor("masks", [128, 128], F16, kind="ExternalInput")
    ident = nc.dram_tensor("ident", [128, 128], BF, kind="ExternalInput")
    out = nc.dram_tensor("out", [T, D], BF, kind="ExternalOutput")

    with TileContextSplitDrain(nc) as tc:
        with (
            tc.tile_pool(name="static", bufs=1) as st,
            tc.tile_pool(name="xt", bufs=2) as xt_pool,
            tc.tile_pool(name="ropef", bufs=2) as ropef,
            tc.tile_pool(name="qknat", bufs=3) as qknat_pool,
            tc.tile_pool(name="pt", bufs=8) as pt_pool,
            tc.tile_pool(name="laccp", bufs=4) as laccp,
            tc.tile_pool(name="small", bufs=2) as small,
            tc.tile_pool(name="ostage", bufs=3) as ostage_pool,
            tc.tile_pool(name="psum_proj", bufs=2, space="PSUM") as ps_proj,
            tc.tile_pool(name="psum_trs", bufs=3, space="PSUM") as ps_trs,
            tc.tile_pool(name="psum_y", bufs=3, space="PSUM") as ps_y,
        ):
            # --- static loads, ordered so the first qk matmul can start
            # as early as possible: wq (4 dt-group DMAs, scalar ring)
            # races x chunk 0 (4 dt-group DMAs, sync ring) ---
            # startup: interleave wq / x-chunk-0 / early rope tables across
            # both DMA rings so the first tile is never starved
            wqv = wqkvT.rearrange("(dt p) e -> p dt e", p=128)
            xbT3 = xbT.rearrange("(dt p) t -> p dt t", p=128)
            wq_sb = st.tile([128, DT, EQK + EV], BF)
            xT_c0 = xt_pool.tile([128, DT, TCH], BF, name="xT_c")
            cos_sb = st.tile([128, LT, dh], FP32)
            sin_sb = st.tile([128, LT, dh], FP32)
            for g in range(8):
                nc.sync.dma_start(
                    xT_c0[:, 2 * g:2 * g + 2, :], xbT3[:, 2 * g:2 * g + 2, 0:TCH]
                )
            nc.scalar.dma_start(wq_sb[:, 0:4, :], wqv[:, 0:4, :])
            nc.scalar.dma_start(cos_sb[:, 0:4, :], cosn[:, 0:4, :])
            nc.scalar.dma_start(sin_sb[:, 0:4, :], sinn[:, 0:4, :])
            nc.scalar.dma_start(wq_sb[:, 4:8, :], wqv[:, 4:8, :])
            nc.scalar.dma_start(wq_sb[:, 8:12, :], wqv[:, 8:12, :])
            nc.scalar.dma_start(wq_sb[:, 12:16, :], wqv[:, 12:16, :])
            ident_sb = st.tile([128, 128], BF)
            nc.scalar.dma_start(ident_sb[:], ident[:])
            nc.scalar.dma_start(cos_sb[:, 4:LT, :], cosn[:, 4:LT, :])
            nc.scalar.dma_start(sin_sb[:, 4:LT, :], sinn[:, 4:LT, :])
            masks_sb = st.tile([128, 128], F16)
            nc.scalar.dma_start(masks_sb[:], masks[:])
            wo_sb = st.tile([128, HL, D], BF)
            nc.scalar.dma_start(wo_sb[:], woT.rearrange("(h p) e -> p h e", p=128))
            ones_mat = st.tile([128, 128], F16)
            nc.vector.memset(ones_mat[:], 1.0)

            qT_sb = st.tile([128, HL, B, L], BF)
            kT_sb = st.tile([128, HL, B, L], BF)
            v_sb = st.tile([128, T // 128, EV], F16)
            yT_sb = st.tile([128, HL, B, L], BF)

            # --- phase A: qkv projection + rope + q/k transpose ---
            copy_state = [0]

            def emit_a_chunk(tch, pump_n=0):
                copy_flip = copy_state[0]
                b = (tch * TCH) // L
                l0 = (tch * TCH) % L
                if tch == 0:
                    xT_c = xT_c0    # loaded in the startup interleave
                else:
                    xT_c = xt_pool.tile([128, DT, TCH], BF, name="xT_c")
                    csl = slice(tch * TCH, (tch + 1) * TCH)
                    # dt-splits keep 1KB DRAM rows per descriptor; early
                    # chunks split across two rings for bandwidth
                    for g in range(2):
                        nc.sync.dma_start(
                            out=xT_c[:, 8 * g:8 * g + 8, :],
                            in_=xbT3[:, 8 * g:8 * g + 8, csl],
                        )
                for tt in range(TPC):
                    lt = (l0 + tt * 128) // 128
                    tstat = [xT_c[:, dt, tt * 128:(tt + 1) * 128] for dt in range(DT)]
                    # q,k natural [t, e]
                    ps_qk = ps_proj.tile([128, EQK], FP32, tag="proj")
                    for dt in range(DT):
                        nc.tensor.matmul(
                            ps_qk[:], tstat[dt], wq_sb[:, dt, 0:EQK],
                            start=(dt == 0), stop=(dt == DT - 1),
                        )
                    # v natural [t, e] (PE does v while rope consumes ps_qk)
                    ps_v = ps_proj.tile([128, EV], FP32, tag="proj")
                    for dt in range(DT):
                        nc.tensor.matmul(
                            ps_v[:], tstat[dt], wq_sb[:, dt, EQK:EQK + EV],
                            start=(dt == 0), stop=(dt == DT - 1),
                        )
                    # rope: the pair swap is folded into tB's read as a
                    # negative-stride view of the psum (no staging copies)
                    cos_b = _bcast_mid(cos_sb[:, lt, :], 2 * HL)
                    sin4 = bass.AP(
                        sin_sb.tensor,
                        sin_sb[:, lt, :].offset,
                        [sin_sb.ap[0], [0, 2 * HL], [2, dh // 2], [1, 2]],
                    )
                    ps4 = ps_qk.rearrange("p (h c two) -> p h c two", h=2 * HL, two=2)
                    sw4 = bass.AP(
                        ps_qk.tensor,
                        ps_qk.offset + 1,
                        [ps_qk.ap[0], [dh, 2 * HL], [2, dh // 2], [-1, 2]],
                    )
                    tA = ropef.tile([128, EQK], FP32, tag="tA")
                    tB = ropef.tile([128, EQK], FP32, tag="tB")
                    nc.vector.tensor_mul(
                        tA.rearrange("p (h e) -> p h e", e=dh),
                        ps_qk.rearrange("p (h e) -> p h e", e=dh), cos_b
                    )
                    nc.vector.tensor_mul(
                        tB.rearrange("p (h c two) -> p h c two", h=2 * HL, two=2),
                        sw4, sin4
                    )
                    qk_nat = qknat_pool.tile([128, EQK], BF)
                    nc.vector.tensor_add(qk_nat[:], tA[:], tB[:])
                    # transpose q,k slices -> qT/kT (PE reaches here after the
                    # v matmuls, by which time qk_nat is ready)
                    ps_t = ps_trs.tile([128, EQK], BF, tag="trs")
                    for j in range(2 * HL):
                        nc.tensor.transpose(
                            ps_t[:, j * 128:(j + 1) * 128],
                            qk_nat[:, j * 128:(j + 1) * 128],
                            ident_sb[:],
                        )
                    lsl = slice(l0 + tt * 128, l0 + (tt + 1) * 128)
                    q_dst = qT_sb[:, :, b, lsl]
                    k_dst = kT_sb[:, :, b, lsl]
                    q_src = ps_t[:, 0:HL * 128].rearrange("p (h t) -> p h t", h=HL)
                    k_src = ps_t[:, HL * 128:2 * HL * 128].rearrange(
                        "p (h t) -> p h t", h=HL
                    )
                    nc.scalar.activation(
                        q_dst, q_src, mybir.ActivationFunctionType.Copy
                    )
                    if tch >= 6:
                        # late chunks: k_dst via scalar so the (1,qc2/qc3)
                        # S matmuls don't wait on the deep vector queue
                        nc.scalar.activation(
                            k_dst, k_src, mybir.ActivationFunctionType.Copy
                        )
                    else:
                        nc.vector.tensor_copy(k_dst, k_src)
                    copy_flip += 1
                    nc.scalar.activation(
                        v_sb[:, tch * TPC + tt, :], ps_v[:],
                        mybir.ActivationFunctionType.Copy,
                    )
                    emit_c_mini()
                    if pump_n:
                        pump(pump_n)
                copy_state[0] = copy_flip

            # --- phases B and C, woven: attention per (b, qc) with both
            # heads block-interleaved; o_proj mini-units (b, lt, ec) for the
            # previous q-chunk fill PE slack and spread the output stores ---
            c_queue = []       # pending o_proj minis: (b, lt, ec)
            c_stage = [None]   # current per-lt staging tile
            c_tail_lt = [None]  # lt whose staging began in drain mode

            def emit_c_mini(tail=False):
                if not c_queue:
                    return
                b, lt, ec = c_queue.pop(0)
                if tail and ec == 0:
                    c_tail_lt[0] = (b, lt)
                tail = tail and c_tail_lt[0] == (b, lt)
                ttg = b * LT + lt
                ps_o = ps_proj.tile([128, 512], FP32, tag="proj")
                for h in range(HL):
                    nc.tensor.matmul(
                        ps_o[:],
                        yT_sb[:, h, b, lt * 128:(lt + 1) * 128],
                        wo_sb[:, h, ec * 512:(ec + 1) * 512],
                        start=(h == 0), stop=(h == HL - 1),
                    )
                if ec == 0:
                    c_stage[0] = ostage_pool.tile([128, D // 512, 512], BF, name="ost")
                ost = c_stage[0]
                # casts split scalar/vector (gpsimd cannot read PSUM);
                # vector-heavy in the tail where scalar still runs exps
                if ec in (0, 2) or (tail and ec == 1):
                    nc.vector.tensor_copy(ost[:, ec, :], ps_o[:])
                else:
                    nc.scalar.activation(
                        ost[:, ec, :], ps_o[:], mybir.ActivationFunctionType.Copy
                    )
                if tail:
                    # in the drain, store each 512-wide slice as soon as its
                    # cast lands instead of batching the whole row
                    nc.sync.dma_start(
                        out[ttg * 128:(ttg + 1) * 128, ec * 512:(ec + 1) * 512],
                        ost[:, ec, :],
                    )
                elif ec == D // 512 - 1:
                    nc.sync.dma_start(
                        out[ttg * 128:(ttg + 1) * 128, :],
                        ost.rearrange("p e c -> p (e c)"),
                    )

            def phase_b_qc(b, qc):
                nk = (qc + 1) * NDIAG
                ps_yt = [
                    ps_y.tile([128, QC], FP32, name=f"ps_yt{h}", tag="y")
                    for h in range(HL)
                ]
                lacc = [
                    laccp.tile([128, QC], F16, name=f"lacc{h}", tag="lacc")
                    for h in range(HL)
                ]
                pT_blk = [[None] * nk for _ in range(HL)]
                cs_blk = [None] * nk

                def s_exp(h, kb):
                    q_lo = max(0, kb * 128 - qc * QC)
                    cs = slice(q_lo, QC)
                    cs_blk[kb] = (q_lo, cs)
                    qmov = qT_sb[:, h, b, qc * QC + q_lo:(qc + 1) * QC]
                    ps_s = ps_trs.tile([128, QC], FP32, tag="trs")
                    nc.tensor.matmul(
                        ps_s[:, cs],
                        kT_sb[:, h, b, kb * 128:(kb + 1) * 128],
                        qmov,
                        start=True, stop=True,
                    )
                    pT = pt_pool.tile([128, QC], F16)
                    nc.scalar.activation(
                        pT[:, cs], ps_s[:, cs],
                        mybir.ActivationFunctionType.Exp, scale=scale,
                    )
                    # launch the diag mask on gpsimd right away: its Q7
                    # launch+sem latency hides behind the one-group lag
                    if kb >= NDIAG * qc:
                        nc.gpsimd.tensor_mul(
                            pT[:, q_lo:q_lo + 128],
                            pT[:, q_lo:q_lo + 128],
                            masks_sb[:],
                        )
                    pT_blk[h][kb] = pT

                def post(h, kb):
                    q_lo, cs = cs_blk[kb]
                    pT = pT_blk[h][kb]
                    if kb == 0:
                        nc.vector.tensor_copy(lacc[h][:], pT[:])
                    else:
                        nc.vector.tensor_add(lacc[h][:, cs], lacc[h][:, cs], pT[:, cs])
                    nc.tensor.matmul(
                        ps_yt[h][:, cs],
                        v_sb[:, b * LT + kb, h * dh:(h + 1) * dh],
                        pT[:, cs],
                        start=(kb == 0), stop=(kb == nk - 1),
                    )

                # software pipeline: o_proj minis pad the gap between the
                # S matmuls for kb and the PV matmuls for kb-1, giving the
                # exp/mask chain time to produce p^T
                for kb in range(nk):
                    for h in range(HL):
                        s_exp(h, kb)
                    emit_c_mini()
                    if len(c_queue) > 20:
                        emit_c_mini()
                    if kb >= 1:
                        for h in range(HL):
                            post(h, kb - 1)
                    yield
                for h in range(HL):
                    post(h, nk - 1)
                # normalization: l = ones^T @ lacc (replicated across
                # partitions), 1/l = exp(-ln l), yT = ps_yt * (1/l)
                for h in range(HL):
                    emit_c_mini()
                    ps_lt = ps_trs.tile([128, QC], FP32, tag="trs", name="ps_lt")
                    nc.tensor.matmul(
                        ps_lt[:], ones_mat[:], lacc[h][:], start=True, stop=True
                    )
                    emit_c_mini()
                    lnl = small.tile([128, QC], FP32, tag="lnl")
                    nc.scalar.activation(
                        lnl[:], ps_lt[:], mybir.ActivationFunctionType.Ln
                    )
                    invb = small.tile([128, QC], FP32, tag="invb")
                    nc.scalar.activation(
                        invb[:], lnl[:],
                        mybir.ActivationFunctionType.Exp, scale=-1.0,
                    )
                    nc.vector.tensor_mul(
                        yT_sb[:, h, b, qc * QC:(qc + 1) * QC], ps_yt[h][:], invb[:]
                    )
                    yield
                # this q-chunk's o_proj minis become available for weaving
                for lt in range(qc * NDIAG, (qc + 1) * NDIAG):
                    for ec in range(D // 512):
                        c_queue.append((b, lt, ec))

            # --- global weave: attention emission is a generator pumped a
            # few steps after every phase-A tile, so A matmuls fill the
            # S->exp->mask->lacc->PV latency chain and the scalar/vector
            # load spreads over the whole kernel. Unit (b, qc) is gated on
            # the A chunk holding its data being fully emitted. ---
            chunks_emitted = [0]

            def b_stream():
                for b in range(B):
                    for qc in range(NQC):
                        yield (1 + qc) if b == 0 else (5 + qc)  # chunks needed
                        yield from phase_b_qc(b, qc)

            gen = b_stream()
            gate = [0]
            done = [False]

            def pump(n):
                for _ in range(n):
                    if done[0] or chunks_emitted[0] < gate[0]:
                        return
                    try:
                        g = next(gen)
                    except StopIteration:
                        done[0] = True
                        return
                    if isinstance(g, int):
                        gate[0] = g

            for tch in range(NCH):
                emit_a_chunk(tch, pump_n=4 if tch >= 1 else 0)
                chunks_emitted[0] = tch + 1
            while not done[0]:
                pump(4)
                emit_c_mini()
                emit_c_mini()
            while c_queue:
                emit_c_mini(tail=True)
    return nc


def _rope_tables(L, dh, LT):
    inv_freq = 1.0 / (ROPE_THETA ** (np.arange(0, dh, 2, dtype=np.float32) / dh))
    ang = np.arange(L, dtype=np.float32)[:, None] * inv_freq[None, :]  # [L, dh/2]
    cos = np.repeat(np.cos(ang), 2, axis=-1)                          # [L, dh]
    sin = np.repeat(np.sin(ang), 2, axis=-1)
    sgn = np.where(np.arange(dh) % 2 == 0, -1.0, 1.0).astype(np.float32)
    sinn = sin * sgn[None, :]
    # [L, dh] -> [128, LT, dh] with partition = l % 128
    cosn = np.ascontiguousarray(
        cos.reshape(LT, 128, dh).transpose(1, 0, 2)
    ).astype(np.float32)
    sinn = np.ascontiguousarray(
        sinn.reshape(LT, 128, dh).transpose(1, 0, 2)
    ).astype(np.float32)
    return cosn, sinn


def make_in_maps(x, W_qkv, W_o, n_cores=8, H=16):
    B, L, D = x.shape
    T = B * L
    dh = D // H
    HL = H // n_cores
    LT = L // 128
    xbfT = np.ascontiguousarray(x.reshape(T, D).T).astype(BF16)
    cosn, sinn = _rope_tables(L, dh, LT)
    p = np.arange(128)[:, None]
    f = np.arange(128)[None, :]
    mask = (p <= f).astype(FP16)
    identity = np.eye(128, dtype=BF16)
    in_maps = []
    for c in range(n_cores):
        r0 = c * HL * dh
        r1 = (c + 1) * HL * dh
        wl = np.concatenate(
            [W_qkv[r0:r1], W_qkv[D + r0:D + r1], W_qkv[2 * D + r0:2 * D + r1]], axis=0
        )
        wqkvT = np.ascontiguousarray(wl.T).astype(BF16)
        woT = np.ascontiguousarray(W_o[:, r0:r1].T).astype(BF16)
        in_maps.append(
            {
                "xbT": xbfT,
                "wqkvT": wqkvT,
                "woT": woT,
                "cosn": cosn,
                "sinn": sinn,
                "masks": mask,
                "ident": identity,
            }
        )
    return in_maps


_NC_CACHE = {}


def _get_nc(B, L, D, HL):
    key = (B, L, D, HL)
    if key not in _NC_CACHE:
        _NC_CACHE[key] = build_core_kernel(B, L, D, HL)
    return _NC_CACHE[key]


def kernel(x, W_qkv, W_o, trace=False):
    x = np.asarray(x)
    W_qkv = np.asarray(W_qkv)
    W_o = np.asarray(W_o)
    B, L, D = x.shape
    n_cores, H = 8, 16
    HL = H // n_cores
    nc = _get_nc(B, L, D, HL)
    in_maps = make_in_maps(x, W_qkv, W_o, n_cores=n_cores, H=H)
    res = run_bass_kernel_spmd(
        nc, in_maps, core_ids=list(range(n_cores)), trace=trace
    )
    acc = np.zeros((B * L, D), dtype=np.float64)
    for r in res.results:
        acc += r["out"].astype(np.float64)
    out = acc.astype(np.float32).reshape(B, L, D)
    if trace:
        return out, res
    return out
